# revision 14
# baseline (speedup 1.0000x reference)
"""MoE expert-combine kernel for Trainium2 (raw Bass, hand-scheduled), 8-core SPMD.

Problem: out[b,s,:] = sum_k expert_weights[b,s,k] * expert_outputs[expert_indices[b,s,k], b, s, :]
  B,S,H = 4,2048,1024 ; E=8 ; K=2  (hidden_states is unused by the reference)

Sharding: flatten tokens t = b*S+s (8192 total); each of the 8 cores owns a
contiguous block of 1024 tokens. Each core receives the expert-output stack
sliced to its tokens ([E, 1024, H] viewed as a row table [E*1024, H]) plus
host-precomputed gather row indices (idx[t,k]*1024 + t_local) and weights,
packed into one [128, 32] int32 tensor (16 idx cols + 16 f32-bitcast weight
cols).

Token->partition map: partition p owns tokens p*8 .. p*8+7 (j = 0..7). This
makes each partition's 8 output rows CONTIGUOUS in DRAM, so stores move 8KB
per descriptor instead of 4KB — fewer descriptors means less SWDGE
descriptor-ring fetch pressure (the known SDMA engine-7/15 straggler).

Per j-slot: two indirect-DMA gathers (Pool/SWDGE) fetch the selected 4KB
table rows for tokens {p*8+j}, DVE combines (w0*g0 via tensor_scalar, then
(w1*g1)+acc via scalar_tensor_tensor), and every 2 slots one HWDGE store
writes [128, 2, 1024] to DRAM. Hand-placed semaphores, at most one sync-wait
per compute instruction (walrus limit), and no end-of-block all-engine
barrier (the sync engine's final sem_st wait covers every data dependency).
"""

import sys
import numpy as np

for _p in ("/opt/trn_rl_repo", "/opt/pypackages"):
    if _p not in sys.path:
        sys.path.append(_p)

from concourse import bass, mybir
from concourse.bass_utils import run_bass_kernel_spmd

B, S, H = 4, 2048, 1024
E, K = 8, 2
N_CORES = 8
T = B * S              # 8192 tokens total
TC = T // N_CORES      # 1024 tokens per core
P = 128                # SBUF partitions
NJ = TC // P           # 8 tokens per partition (j slots)
ST_J = 2               # j slots per output store
N_ST = NJ // ST_J      # 4 stores

_f32 = mybir.dt.float32
_i32 = mybir.dt.int32


def _build():
    # Two SWDGE queues: their descriptor rings live on different partition
    # groups, splitting the ring-fetch port pressure that makes SDMA engines
    # 7/15 straggle when everything runs through one ring.
    nc = bass.Bass(target_bir_lowering=False, num_swdge_queues=2)

    table = nc.declare_dram_parameter("table", [E * TC, H], _f32, isOutput=False)
    idxw = nc.declare_dram_parameter("idxw", [P, 2 * NJ * K], _i32, isOutput=False)
    out = nc.declare_dram_parameter("out", [TC, H], _f32, isOutput=True)

    with (
        nc.semaphore("sem_in") as sem_in,
        nc.semaphore("sem_v") as sem_v,
        nc.semaphore("sem_st") as sem_st,
        nc.sbuf_tensor("idxw_t", [P, 2 * NJ * K], _i32) as idxw_t,
        nc.sbuf_tensor("g_t", [P, NJ * K * H], _f32) as g_t,
        nc.sbuf_tensor("ot_t", [P, NJ * H], _f32) as ot_t,
        nc.sbuf_tensor("acc_t", [P, H], _f32) as acc_t,
    ):
        gather_sems = [nc.alloc_semaphore(f"sem_g{i}") for i in range(NJ * K)]
        blk = bass.BassBlock(nc, "main")
        nc.cur_block = blk

        def sync_body(sync: bass.BassEngine):
            sync.dma_start(out=idxw_t[:], in_=idxw[:]).then_inc(sem_in, 16)
            out_v = out.ap().rearrange("(p j) h -> p j h", p=P)  # [128, 8, 1024]
            for s in range(N_ST):
                # store s covers j slots {2s, 2s+1}: ready after DVE op 4s+4
                sync.wait_ge(sem_v, 4 * s + 4)
                sync.dma_start(
                    out=out_v[:, ST_J * s : ST_J * (s + 1), :],
                    in_=ot_t[:, ST_J * s * H : ST_J * (s + 1) * H],
                ).then_inc(sem_st, 16)
            sync.wait_ge(sem_st, 16 * N_ST)

        def gpsimd_body(gpsimd: bass.BassEngine):
            gpsimd.wait_ge(sem_in, 16)
            for j in range(NJ):
                for k in range(K):
                    m = j * K + k
                    inst = gpsimd.indirect_dma_start(
                        out=g_t[:, m * H : (m + 1) * H],
                        out_offset=None,
                        in_=table[:],
                        in_offset=bass.IndirectOffsetOnAxis(
                            ap=idxw_t[:, m : m + 1], axis=0
                        ),
                    ).then_inc(gather_sems[m], 16)
                    if m % 2 == 1:
                        inst.ins.queue = "qPoolDynamic1"

        def vector_body(vector: bass.BassEngine):
            for j in range(NJ):
                m0, m1 = j * K, j * K + 1
                w0 = idxw_t[:, NJ * K + m0 : NJ * K + m0 + 1].bitcast(_f32)
                w1 = idxw_t[:, NJ * K + m1 : NJ * K + m1 + 1].bitcast(_f32)
                # w cols arrive in the same DMA the gathers waited on, so a
                # completed gather implies the weights are resident too.
                vector.tensor_scalar(
                    out=acc_t[:],
                    in0=g_t[:, m0 * H : (m0 + 1) * H],
                    scalar1=w0,
                    scalar2=None,
                    op0=mybir.AluOpType.mult,
                )._wait_ge(gather_sems[m0], 16).then_inc(sem_v, 1)
                vector.scalar_tensor_tensor(
                    out=ot_t[:, j * H : (j + 1) * H],
                    in0=g_t[:, m1 * H : (m1 + 1) * H],
                    scalar=w1,
                    in1=acc_t[:],
                    op0=mybir.AluOpType.mult,
                    op1=mybir.AluOpType.add,
                )._wait_ge(gather_sems[m1], 16).then_inc(sem_v, 1)

        blk.sync(sync_body)
        blk.gpsimd(gpsimd_body)
        blk.vector(vector_body)

        # Manual block exit WITHOUT the end-of-block drains + all-engine
        # barrier (~5us of pure tail): branch every engine to the end block.
        for engine, last_body in blk.last_body.items():
            with nc.body(last_body, parent=nc.cur_bb, allow_existing_parent=True):
                engine.br(blk.end_bb)
        nc.switch_bb(blk.end_bb)
        nc.cur_block = None

    nc.finalize()
    return nc


def _prepare_in_maps(expert_indices, expert_weights, expert_outputs):
    eo = np.ascontiguousarray(np.asarray(expert_outputs, dtype=np.float32)).reshape(
        E, T, H
    )
    flat_idx = np.asarray(expert_indices).reshape(T, K).astype(np.int32)
    flat_w = np.asarray(expert_weights, dtype=np.float32).reshape(T, K)
    t_local = np.arange(TC, dtype=np.int32)[:, None]
    in_maps = []
    for i in range(N_CORES):
        t0 = i * TC
        slab = np.ascontiguousarray(eo[:, t0 : t0 + TC, :]).reshape(E * TC, H)
        li = flat_idx[t0 : t0 + TC] * TC + t_local  # [TC, K] row idx into slab
        # partition p owns tokens p*8 + j  ->  [P, NJ*K] with col j*K+k
        li = li.reshape(P, NJ * K)
        w = flat_w[t0 : t0 + TC].reshape(P, NJ * K)
        idxw = np.empty((P, 2 * NJ * K), dtype=np.int32)
        idxw[:, : NJ * K] = li
        idxw[:, NJ * K :] = np.ascontiguousarray(w.astype(np.float32)).view(np.int32)
        in_maps.append({"table": slab, "idxw": idxw})
    return in_maps


def run(
    hidden_states,
    expert_indices,
    expert_weights,
    expert_outputs,
    trace=False,
):
    in_maps = _prepare_in_maps(expert_indices, expert_weights, expert_outputs)
    nc = _build()
    res = run_bass_kernel_spmd(nc, in_maps, list(range(N_CORES)), trace=trace)
    outs = [np.asarray(res.results[i]["out"]) for i in range(N_CORES)]
    full = np.concatenate(outs, axis=0).reshape(B, S, H).astype(np.float32)
    return full, res


def kernel(hidden_states, expert_indices, expert_weights, expert_outputs):
    full, _ = run(hidden_states, expert_indices, expert_weights, expert_outputs)
    return full


# revision 15
# speedup vs baseline: 1.0876x; 1.0876x over previous
"""MoE expert-combine kernel for Trainium2 (raw Bass, hand-scheduled), 8-core SPMD.

Problem: out[b,s,:] = sum_k expert_weights[b,s,k] * expert_outputs[expert_indices[b,s,k], b, s, :]
  B,S,H = 4,2048,1024 ; E=8 ; K=2  (hidden_states is unused by the reference)

Sharding: flatten tokens t = b*S+s (8192 total); each of the 8 cores owns a
contiguous block of 1024 tokens. Each core receives the expert-output stack
sliced to its tokens ([E, 1024, H] viewed as a row table [E*1024, H]) plus
host-precomputed gather row indices (idx[t,k]*1024 + t_local, int32
[128, 16]) and weights (f32 [128, 16]).

On-device, per 128-token chunk c (token = c*128 + p): two indirect-DMA
gathers (Pool/SWDGE) fetch the selected 4KB table rows, DVE combines
(w0*g0 via tensor_scalar, then (w1*g1)+acc via scalar_tensor_tensor), and an
HWDGE store writes [128, 1024] back. The row-index tensor is loaded in its
own small DMA ahead of the weights so the first gather can start as early as
possible. Hand-placed semaphores, at most one sync-wait per compute
instruction (walrus codegen limit), and no end-of-block drain/barrier (the
sync engine's final sem_st wait covers every data dependency; the NEFF's own
per-engine completion chain runs regardless).
"""

import sys
import numpy as np

for _p in ("/opt/trn_rl_repo", "/opt/pypackages"):
    if _p not in sys.path:
        sys.path.append(_p)

from concourse import bass, mybir
from concourse.bass_utils import run_bass_kernel_spmd

B, S, H = 4, 2048, 1024
E, K = 8, 2
N_CORES = 8
T = B * S              # 8192 tokens total
TC = T // N_CORES      # 1024 tokens per core
P = 128                # SBUF partitions
NCHUNK = TC // P       # 8 chunks of 128 tokens per core

_f32 = mybir.dt.float32
_i32 = mybir.dt.int32


def _build():
    nc = bass.Bass(target_bir_lowering=False)

    table = nc.declare_dram_parameter("table", [E * TC, H], _f32, isOutput=False)
    idx = nc.declare_dram_parameter("idx", [P, NCHUNK * K], _i32, isOutput=False)
    wgt = nc.declare_dram_parameter("wgt", [P, NCHUNK * K], _f32, isOutput=False)
    out = nc.declare_dram_parameter("out", [TC, H], _f32, isOutput=True)

    with (
        nc.semaphore("sem_idx") as sem_idx,
        nc.semaphore("sem_w") as sem_w,
        nc.semaphore("sem_v") as sem_v,
        nc.semaphore("sem_st") as sem_st,
        nc.sbuf_tensor("idx_t", [P, NCHUNK * K], _i32) as idx_t,
        nc.sbuf_tensor("w_t", [P, NCHUNK * K], _f32) as w_t,
        nc.sbuf_tensor("g_t", [P, NCHUNK * K * H], _f32) as g_t,
        nc.sbuf_tensor("ot_t", [P, NCHUNK * H], _f32) as ot_t,
        nc.sbuf_tensor("acc_t", [P, H], _f32) as acc_t,
    ):
        gather_sems = [nc.alloc_semaphore(f"sem_g{i}") for i in range(NCHUNK * K)]
        blk = bass.BassBlock(nc, "main")
        nc.cur_block = blk

        def sync_body(sync: bass.BassEngine):
            sync.dma_start(out=idx_t[:], in_=idx[:]).then_inc(sem_idx, 16)
            sync.dma_start(out=w_t[:], in_=wgt[:]).then_inc(sem_w, 16)
            for c in range(NCHUNK):
                # ot chunk c is ready after DVE op 2c+2 (1 sem inc per op)
                sync.wait_ge(sem_v, 2 * c + 2)
                sync.dma_start(
                    out=out[c * P : (c + 1) * P, :],
                    in_=ot_t[:, c * H : (c + 1) * H],
                ).then_inc(sem_st, 16)
            sync.wait_ge(sem_st, 16 * NCHUNK)

        def gpsimd_body(gpsimd: bass.BassEngine):
            gpsimd.wait_ge(sem_idx, 16)
            for c in range(NCHUNK):
                for k in range(K):
                    m = c * K + k
                    gpsimd.indirect_dma_start(
                        out=g_t[:, m * H : (m + 1) * H],
                        out_offset=None,
                        in_=table[:],
                        in_offset=bass.IndirectOffsetOnAxis(
                            ap=idx_t[:, m : m + 1], axis=0
                        ),
                    ).then_inc(gather_sems[m], 16)

        def vector_body(vector: bass.BassEngine):
            # one-time gate on the weight load; afterwards each op's single
            # wait slot is spent on its gather sem
            vector.wait_ge(sem_w, 16)
            for c in range(NCHUNK):
                m0, m1 = c * K, c * K + 1
                w0 = w_t[:, m0 : m0 + 1]
                w1 = w_t[:, m1 : m1 + 1]
                vector.tensor_scalar(
                    out=acc_t[:],
                    in0=g_t[:, m0 * H : (m0 + 1) * H],
                    scalar1=w0,
                    scalar2=None,
                    op0=mybir.AluOpType.mult,
                )._wait_ge(gather_sems[m0], 16).then_inc(sem_v, 1)
                vector.scalar_tensor_tensor(
                    out=ot_t[:, c * H : (c + 1) * H],
                    in0=g_t[:, m1 * H : (m1 + 1) * H],
                    scalar=w1,
                    in1=acc_t[:],
                    op0=mybir.AluOpType.mult,
                    op1=mybir.AluOpType.add,
                )._wait_ge(gather_sems[m1], 16).then_inc(sem_v, 1)

        blk.sync(sync_body)
        blk.gpsimd(gpsimd_body)
        blk.vector(vector_body)

        # Manual block exit WITHOUT the end-of-block drains + all-engine
        # barrier: branch every engine straight to the end block.
        for engine, last_body in blk.last_body.items():
            with nc.body(last_body, parent=nc.cur_bb, allow_existing_parent=True):
                engine.br(blk.end_bb)
        nc.switch_bb(blk.end_bb)
        nc.cur_block = None

    nc.finalize()
    return nc


def _prepare_in_maps(expert_indices, expert_weights, expert_outputs):
    eo = np.ascontiguousarray(np.asarray(expert_outputs, dtype=np.float32)).reshape(
        E, T, H
    )
    flat_idx = np.asarray(expert_indices).reshape(T, K).astype(np.int32)
    flat_w = np.asarray(expert_weights, dtype=np.float32).reshape(T, K)
    t_local = np.arange(TC, dtype=np.int32)[:, None]
    in_maps = []
    for i in range(N_CORES):
        t0 = i * TC
        slab = np.ascontiguousarray(eo[:, t0 : t0 + TC, :]).reshape(E * TC, H)
        li = flat_idx[t0 : t0 + TC] * TC + t_local  # [TC, K] row idx into slab
        # chunk-major: partition p of chunk c holds token c*128+p
        li = np.ascontiguousarray(
            li.reshape(NCHUNK, P, K).transpose(1, 0, 2).reshape(P, NCHUNK * K)
        )
        w = np.ascontiguousarray(
            flat_w[t0 : t0 + TC]
            .reshape(NCHUNK, P, K)
            .transpose(1, 0, 2)
            .reshape(P, NCHUNK * K)
            .astype(np.float32)
        )
        in_maps.append({"table": slab, "idx": li, "wgt": w})
    return in_maps


def run(
    hidden_states,
    expert_indices,
    expert_weights,
    expert_outputs,
    trace=False,
):
    in_maps = _prepare_in_maps(expert_indices, expert_weights, expert_outputs)
    nc = _build()
    res = run_bass_kernel_spmd(nc, in_maps, list(range(N_CORES)), trace=trace)
    outs = [np.asarray(res.results[i]["out"]) for i in range(N_CORES)]
    full = np.concatenate(outs, axis=0).reshape(B, S, H).astype(np.float32)
    return full, res


def kernel(hidden_states, expert_indices, expert_weights, expert_outputs):
    full, _ = run(hidden_states, expert_indices, expert_weights, expert_outputs)
    return full


# revision 16
# speedup vs baseline: 1.1931x; 1.0970x over previous
"""MoE expert-combine kernel for Trainium2 (raw Bass, hand-scheduled), 8-core SPMD.

Problem: out[b,s,:] = sum_k expert_weights[b,s,k] * expert_outputs[expert_indices[b,s,k], b, s, :]
  B,S,H = 4,2048,1024 ; E=8 ; K=2  (hidden_states is unused by the reference)

Sharding: flatten tokens t = b*S+s (8192 total); each of the 8 cores owns a
contiguous block of 1024 tokens. Each core receives the expert-output stack
sliced to its tokens ([E, 1024, H] viewed as a row table [E*1024, H]) plus
host-precomputed gather row indices (idx[t,k]*1024 + t_local, int32
[128, 16]) and weights (f32 [128, 16]).

On-device, per 128-token chunk c (token = c*128 + p): two indirect-DMA
gathers (Pool/SWDGE) fetch the selected 4KB table rows, DVE combines
(w0*g0 via tensor_scalar, then (w1*g1)+acc via scalar_tensor_tensor), and an
HWDGE store writes [128, 1024] back. The row-index tensor is loaded in its
own small DMA ahead of the weights so the first gather can start as early as
possible. Hand-placed semaphores, at most one sync-wait per compute
instruction (walrus codegen limit), and no end-of-block drain/barrier (the
sync engine's final sem_st wait covers every data dependency; the NEFF's own
per-engine completion chain runs regardless).
"""

import sys
import numpy as np

for _p in ("/opt/trn_rl_repo", "/opt/pypackages"):
    if _p not in sys.path:
        sys.path.append(_p)

from concourse import bass, mybir
from concourse.bass_utils import run_bass_kernel_spmd

B, S, H = 4, 2048, 1024
E, K = 8, 2
N_CORES = 8
T = B * S              # 8192 tokens total
TC = T // N_CORES      # 1024 tokens per core
P = 128                # SBUF partitions
NCHUNK = TC // P       # 8 chunks of 128 tokens per core

_f32 = mybir.dt.float32
_i32 = mybir.dt.int32


def _build():
    nc = bass.Bass(target_bir_lowering=False)

    table = nc.declare_dram_parameter("table", [E * TC, H], _f32, isOutput=False)
    idx = nc.declare_dram_parameter("idx", [P, NCHUNK * K], _i32, isOutput=False)
    wgt = nc.declare_dram_parameter("wgt", [P, NCHUNK * K], _f32, isOutput=False)
    out = nc.declare_dram_parameter("out", [TC, H], _f32, isOutput=True)

    with (
        nc.semaphore("sem_idx") as sem_idx,
        nc.semaphore("sem_w") as sem_w,
        nc.semaphore("sem_v") as sem_v,
        nc.semaphore("sem_st") as sem_st,
        nc.sbuf_tensor("idx_t", [P, NCHUNK * K], _i32) as idx_t,
        nc.sbuf_tensor("w_t", [P, NCHUNK * K], _f32) as w_t,
        nc.sbuf_tensor("g_t", [P, NCHUNK * K * H], _f32) as g_t,
        nc.sbuf_tensor("ot_t", [P, NCHUNK * H], _f32) as ot_t,
        nc.sbuf_tensor("acc_t", [P, H], _f32) as acc_t,
    ):
        gather_sems = [nc.alloc_semaphore(f"sem_g{i}") for i in range(NCHUNK * K)]
        blk = bass.BassBlock(nc, "main")
        nc.cur_block = blk

        def sync_body(sync: bass.BassEngine):
            sync.dma_start(out=idx_t[:], in_=idx[:]).then_inc(sem_idx, 16)
            sync.dma_start(out=w_t[:], in_=wgt[:]).then_inc(sem_w, 16)
            for c in range(NCHUNK):
                # ot chunk c is ready after DVE op 2c+2 (1 sem inc per op)
                sync.wait_ge(sem_v, 2 * c + 2)
                sync.dma_start(
                    out=out[c * P : (c + 1) * P, :],
                    in_=ot_t[:, c * H : (c + 1) * H],
                ).then_inc(sem_st, 16)
            sync.wait_ge(sem_st, 16 * NCHUNK)

        def gpsimd_body(gpsimd: bass.BassEngine):
            gpsimd.wait_ge(sem_idx, 16)
            for c in range(NCHUNK):
                for k in range(K):
                    m = c * K + k
                    gpsimd.indirect_dma_start(
                        out=g_t[:, m * H : (m + 1) * H],
                        out_offset=None,
                        in_=table[:],
                        in_offset=bass.IndirectOffsetOnAxis(
                            ap=idx_t[:, m : m + 1], axis=0
                        ),
                    ).then_inc(gather_sems[m], 16)

        def vector_body(vector: bass.BassEngine):
            # one-time gate on the weight load; afterwards each op's single
            # wait slot is spent on its gather sem
            vector.wait_ge(sem_w, 16)
            for c in range(NCHUNK):
                m0, m1 = c * K, c * K + 1
                w0 = w_t[:, m0 : m0 + 1]
                w1 = w_t[:, m1 : m1 + 1]
                vector.tensor_scalar(
                    out=acc_t[:],
                    in0=g_t[:, m0 * H : (m0 + 1) * H],
                    scalar1=w0,
                    scalar2=None,
                    op0=mybir.AluOpType.mult,
                )._wait_ge(gather_sems[m0], 16).then_inc(sem_v, 1)
                vector.scalar_tensor_tensor(
                    out=ot_t[:, c * H : (c + 1) * H],
                    in0=g_t[:, m1 * H : (m1 + 1) * H],
                    scalar=w1,
                    in1=acc_t[:],
                    op0=mybir.AluOpType.mult,
                    op1=mybir.AluOpType.add,
                )._wait_ge(gather_sems[m1], 16).then_inc(sem_v, 1)

        blk.sync(sync_body)
        blk.gpsimd(gpsimd_body)
        blk.vector(vector_body)

        # Manual block exit WITHOUT the end-of-block drains + all-engine
        # barrier: branch every engine straight to the end block.
        for engine, last_body in blk.last_body.items():
            with nc.body(last_body, parent=nc.cur_bb, allow_existing_parent=True):
                engine.br(blk.end_bb)
        nc.switch_bb(blk.end_bb)
        nc.cur_block = None

    # Strip the preamble's const-tile memsets and the post-init all-engine
    # barrier (~2.5us): this kernel never reads the const APs, and each
    # engine's register init precedes its user code in program order anyway.
    entry = nc.m.functions[0].blocks[0]
    drop = {
        ins.name
        for ins in entry.instructions
        if type(ins).__name__ in ("InstMemset", "InstDrain", "InstEventSemaphore")
    }
    kept = [ins for ins in entry.instructions if ins.name not in drop]
    del entry.instructions[:]
    for ins in kept:
        entry.instructions.append(ins)

    nc.finalize()
    return nc


def _prepare_in_maps(expert_indices, expert_weights, expert_outputs):
    eo = np.ascontiguousarray(np.asarray(expert_outputs, dtype=np.float32)).reshape(
        E, T, H
    )
    flat_idx = np.asarray(expert_indices).reshape(T, K).astype(np.int32)
    flat_w = np.asarray(expert_weights, dtype=np.float32).reshape(T, K)
    t_local = np.arange(TC, dtype=np.int32)[:, None]
    in_maps = []
    for i in range(N_CORES):
        t0 = i * TC
        slab = np.ascontiguousarray(eo[:, t0 : t0 + TC, :]).reshape(E * TC, H)
        li = flat_idx[t0 : t0 + TC] * TC + t_local  # [TC, K] row idx into slab
        # chunk-major: partition p of chunk c holds token c*128+p
        li = np.ascontiguousarray(
            li.reshape(NCHUNK, P, K).transpose(1, 0, 2).reshape(P, NCHUNK * K)
        )
        w = np.ascontiguousarray(
            flat_w[t0 : t0 + TC]
            .reshape(NCHUNK, P, K)
            .transpose(1, 0, 2)
            .reshape(P, NCHUNK * K)
            .astype(np.float32)
        )
        in_maps.append({"table": slab, "idx": li, "wgt": w})
    return in_maps


def run(
    hidden_states,
    expert_indices,
    expert_weights,
    expert_outputs,
    trace=False,
):
    in_maps = _prepare_in_maps(expert_indices, expert_weights, expert_outputs)
    nc = _build()
    res = run_bass_kernel_spmd(nc, in_maps, list(range(N_CORES)), trace=trace)
    outs = [np.asarray(res.results[i]["out"]) for i in range(N_CORES)]
    full = np.concatenate(outs, axis=0).reshape(B, S, H).astype(np.float32)
    return full, res


def kernel(hidden_states, expert_indices, expert_weights, expert_outputs):
    full, _ = run(hidden_states, expert_indices, expert_weights, expert_outputs)
    return full
